# revision 40
# baseline (speedup 1.0000x reference)
"""GQA attention (RoPE, causal) for one TRN2 chip (8 NeuronCores).

Sharding: core d handles batch b = d//4 and kv-group g = d%4
(8 q heads + 1 kv head per core).  Each core computes its partial
output-projection contribution out_partial[b] (shape [S, H]); the host
sums the 4 partials per batch and adds bo.  No collectives.

Per-core layout (all matmul contractions on partitions, all bf16):
  xT   [H, S]       : x[b]^T, loaded chunk-by-chunk (si) double-buffered
  QT   [512, S]     : q^T head-major (rope'd), PERM64 row layout
  KT2  [128, S]     : k^T rope'd (PERM64 rows), duplicated in both halves
  V    [sj,64]+1s   : v transposed with an appended ones column
  scores^T [sj,si]  : lhsT=KT2 block, rhs=QT chunk; diagonal blocks get
                      trimmed si widths packed into two psum tiles
  exp (ACT, scale=1/8; no max-subtraction: |s/8| <~ 6 so exp is safe)
  causal: triangle mask multiply on diag blocks only (DVE, strided view)
  PV: lhsT=[V|1] [sj,65], rhs=expS^T -> psum [65, si] = [attn^T; denom]
  divide: reciprocal_approx (DVE) -> partition_broadcast (Pool) -> mul
  out[s,o]: lhsT=attnT tiles, rhs=woT tiles, psum->sbuf copy, DMA out.

RoPE trick: head-dim rows are stored in PERM64 order (pair (d, d+32) in
adjacent rows), so rotate-half is an adjacent-row swap that a single
DVE stream_shuffle can do (shuffles are quadrant-local).  Wq/Wk rows,
bq/bk, and the cos/sin tables are permuted on the host; QK dot products
are invariant since q and k share the permutation.
"""

import sys

if "/opt/trn_rl_repo" not in sys.path:
    sys.path.insert(0, "/opt/trn_rl_repo")

import numpy as np
import ml_dtypes

bf16 = ml_dtypes.bfloat16

B = 2
S = 2048
H = 2048
N_HEADS = 32
KV_HEADS = 4
HEAD_DIM = 64
ROPE_THETA = 10000.0
N_CORES = 8
ML = 512          # q-head features per core (8 heads * 64)
CHUNK = 512       # si chunk width
HB = 128          # contraction tile

SWAPADJ = [g ^ 1 for g in range(32)]
PERM64 = [(r >> 1) + 32 * (r & 1) for r in range(64)]   # row r <- dim d(r)


def build_graph(nc, tile_mod, mybir, seq=S):
    fp32 = mybir.dt.float32
    bfl = mybir.dt.bfloat16

    nC = seq // CHUNK       # si chunks
    nJ = seq // 128         # sj blocks
    nHB = H // HB           # contraction tiles
    nMT = ML // 128         # q-feature partition tiles (head pairs)

    xT = nc.dram_tensor("xT", [H, seq], bfl, kind="ExternalInput")
    wq = nc.dram_tensor("wq", [H, ML], bfl, kind="ExternalInput")
    wkv = nc.dram_tensor("wkv", [H, 128], bfl, kind="ExternalInput")
    wo = nc.dram_tensor("wo", [ML, H], bfl, kind="ExternalInput")
    bq = nc.dram_tensor("bq", [128, nMT], fp32, kind="ExternalInput")
    bkv = nc.dram_tensor("bkv", [128, 1], fp32, kind="ExternalInput")
    cos2 = nc.dram_tensor("cos2", [128, seq], bfl, kind="ExternalInput")
    sinS = nc.dram_tensor("sinS", [128, seq], bfl, kind="ExternalInput")
    tri = nc.dram_tensor("tri", [128, 2, 128], bfl, kind="ExternalInput")
    id64 = nc.dram_tensor("id64", [64, 64], fp32, kind="ExternalInput")
    out = nc.dram_tensor("out", [seq, H], bfl, kind="ExternalOutput")
    DBG = getattr(build_graph, "_debug", False)
    if DBG:
        dkt2 = nc.dram_tensor("dkt2", [128, seq], bfl, kind="ExternalOutput")
        dqt0 = nc.dram_tensor("dqt0", [128, seq], bfl, kind="ExternalOutput")
        dvt = nc.dram_tensor("dvt", [64, seq], fp32, kind="ExternalOutput")
        dvo0 = nc.dram_tensor("dvo0", [128, 65], bfl, kind="ExternalOutput")
        dat0 = nc.dram_tensor("dat0", [128, seq], bfl, kind="ExternalOutput")
        dew = nc.dram_tensor("dew", [128, 1024], bfl, kind="ExternalOutput")
        dkt2a = nc.dram_tensor("dkt2a", [128, seq], bfl, kind="ExternalOutput")
        dvoa = nc.dram_tensor("dvoa", [128, 65], bfl, kind="ExternalOutput")
        dpv = nc.dram_tensor("dpv", [65, 512], fp32, kind="ExternalOutput")
        drf = nc.dram_tensor("drf", [1, 512], fp32, kind="ExternalOutput")
        dbcs = nc.dram_tensor("dbcs", [64, 512], fp32, kind="ExternalOutput")

    Exp = mybir.ActivationFunctionType.Exp
    tc = tile_mod.TileContext(nc)
    with tc:
        with tc.tile_pool(name="persist", bufs=1) as P, \
             tc.tile_pool(name="xpool", bufs=2) as XP, \
             tc.tile_pool(name="qkps", bufs=2, space="PSUM") as QKP, \
             tc.tile_pool(name="pvps", bufs=2, space="PSUM") as PVP, \
             tc.tile_pool(name="ps512", bufs=2, space="PSUM") as PS5, \
             tc.tile_pool(name="expp", bufs=16) as EP, \
             tc.tile_pool(name="small", bufs=4) as SM, \
             tc.tile_pool(name="rope", bufs=3) as RP, \
             tc.tile_pool(name="outb", bufs=3) as OB:
            wq_t = [P.tile([128, ML], bfl, tag=f"wq{h}", name=f"wq{h}")
                    for h in range(nHB)]
            wkv_t = [P.tile([128, 128], bfl, tag=f"wkv{h}", name=f"wkv{h}")
                     for h in range(nHB)]
            wo_t = [P.tile([128, H], bfl, tag=f"wo{mt}", name=f"wo{mt}")
                    for mt in range(nMT)]
            qt = [P.tile([128, seq], bfl, tag=f"qt{mt}", name=f"qt{mt}")
                  for mt in range(nMT)]
            at = [P.tile([128, seq], bfl, tag=f"at{mt}", name=f"at{mt}")
                  for mt in range(nMT)]
            kt2 = P.tile([128, seq], bfl, tag="kt2", name="kt2")
            vt_sb = P.tile([64, seq], fp32, tag="vt", name="vt_sb")
            vones = [P.tile([128, 65], bfl, tag=f"vo{j}", name=f"vo{j}")
                     for j in range(nJ)]
            cos_t = P.tile([128, seq], bfl, tag="cos", name="cos_t")
            sin_t = P.tile([128, seq], bfl, tag="sin", name="sin_t")
            tri_t = P.tile([128, 2, 128], bfl, tag="tri", name="tri_t")
            bq_t = P.tile([128, nMT], fp32, tag="bq", name="bq_t")
            bkv_t = P.tile([128, 1], fp32, tag="bkv", name="bkv_t")
            id64_t = P.tile([64, 64], fp32, tag="id64", name="id64_t")

            x_tiles = {}

            def emit_x(c):
                cs = slice(CHUNK * c, CHUNK * (c + 1))
                tiles = []
                for h in range(nHB):
                    t = XP.tile([128, CHUNK], bfl, tag=f"x{h}", name=f"x{h}")
                    nc.sync.dma_start(t[:], xT.ap()[128 * h:128 * h + 128, cs])
                    tiles.append(t)
                x_tiles[c] = tiles

            # ---- input DMAs (front-loaded) ------------------------------
            for h in range(nHB):
                nc.sync.dma_start(wkv_t[h][:], wkv.ap()[128 * h:128 * h + 128, :])
            emit_x(0)
            nc.sync.dma_start(cos_t[:], cos2.ap())
            nc.sync.dma_start(sin_t[:], sinS.ap())
            nc.sync.dma_start(bkv_t[:], bkv.ap())
            nc.sync.dma_start(bq_t[:], bq.ap())
            nc.sync.dma_start(tri_t[:], tri.ap())
            nc.sync.dma_start(id64_t[:], id64.ap())
            for h in range(nHB):
                nc.sync.dma_start(wq_t[h][:], wq.ap()[128 * h:128 * h + 128, :])
            if nC > 1:
                emit_x(1)
            for mt in range(nMT):
                nc.sync.dma_start(wo_t[mt][:], wo.ap()[128 * mt:128 * mt + 128, :])

            def rope(ps, bias_col, cs, dst, nrows):
                """RoPE a [128,CHUNK] psum tile into dst (bf16).
                nrows=128 ropes both 64-blocks (Q); nrows=64 ropes rows
                0:64 only (K; rows 64:128 pass through as V via t0)."""
                t0 = RP.tile([128, CHUNK], bfl, tag="t0", name="t0")
                nc.vector.tensor_scalar_add(t0[:], ps[:], bias_col)
                rs = RP.tile([128, CHUNK], bfl, tag="rs", name="rs")
                nc.vector.tensor_mul(rs[0:nrows, :], t0[0:nrows, :],
                                     sin_t[0:nrows, cs])
                sh = RP.tile([128, CHUNK], bfl, tag="sh", name="sh")
                nc.vector.stream_shuffle(sh[0:nrows, :], rs[0:nrows, :],
                                         SWAPADJ)
                m1 = RP.tile([128, CHUNK], bfl, tag="m1", name="m1")
                nc.vector.tensor_mul(m1[0:nrows, :], t0[0:nrows, :],
                                     cos_t[0:nrows, cs])
                nc.vector.tensor_add(dst, m1[0:nrows, :], sh[0:nrows, :])
                return t0

            def emit_kv(c):
                cs = slice(CHUNK * c, CHUNK * (c + 1))
                xc = x_tiles[c]
                ps = PS5.tile([128, CHUNK], fp32, tag="p5", name="pskv")
                for h in range(nHB):
                    nc.tensor.matmul(ps[:], wkv_t[h][:], xc[h][:],
                                     start=(h == 0), stop=(h == nHB - 1))
                t0 = rope(ps, bkv_t[:, 0:1], cs, kt2[0:64, cs], 64)
                nc.gpsimd.tensor_copy(kt2[64:128, cs], kt2[0:64, cs])
                nc.gpsimd.tensor_copy(vt_sb[:, cs], t0[64:128, :])
                # V transposes for this chunk's sj blocks (fp32 so the
                # transpose can share the fp32 psum pool)
                for j in range(4 * c, 4 * c + 4):
                    pv = PS5.tile([128, CHUNK], fp32, tag="p5", name="psv")
                    nc.tensor.transpose(pv[:, 0:64],
                                        vt_sb[:, 128 * j:128 * j + 128],
                                        id64_t[:])
                    nc.gpsimd.memset(vones[j][:, 64:65], 1.0)
                    nc.scalar.copy(vones[j][:, 0:64], pv[:, 0:64])

            def emit_q(c, mt):
                cs = slice(CHUNK * c, CHUNK * (c + 1))
                ms = slice(128 * mt, 128 * mt + 128)
                xc = x_tiles[c]
                ps = PS5.tile([128, CHUNK], fp32, tag="p5", name="psq")
                for h in range(nHB):
                    nc.tensor.matmul(ps[:], wq_t[h][:, ms], xc[h][:],
                                     start=(h == 0), stop=(h == nHB - 1))
                rope(ps, bq_t[:, mt:mt + 1], cs, qt[mt][:, cs], 128)

            # diag pack: D1 holds jb=4c+0 (w=512 @0) and 4c+1 (w=384 @512);
            # D2 holds jb=4c+2 (w=256 @0) and 4c+3 (w=128 @256).
            D_OFF = [(0, 512), (512, 384), (0, 256), (256, 128)]

            def emit_qk(c, mt):
                """QK matmuls + exp (+ diag masks) for both heads of mt.
                Returns per-hh list of (kind, ew) tiles."""
                cs = slice(CHUNK * c, CHUNK * (c + 1))
                ews = [[], []]
                for hh, pbase in ((0, 0), (1, 64)):
                    qsl = slice(pbase, pbase + 64)
                    for p in range(2 * c):   # off-diagonal pair tiles
                        qw = QKP.tile([128, 2 * CHUNK], fp32, tag="qk",
                                      name="qw")
                        for i, jb in enumerate((2 * p, 2 * p + 1)):
                            js = slice(128 * jb, 128 * jb + 128)
                            nc.tensor.matmul(
                                qw[:, CHUNK * i:CHUNK * (i + 1)],
                                kt2[qsl, js], qt[mt][qsl, cs],
                                start=True, stop=True,
                                tile_position=(pbase, 0))
                        ew = EP.tile([128, 2 * CHUNK], bfl, tag="e", name="ew")
                        nc.scalar.activation(ew[:], qw[:], Exp, scale=0.125)
                        ews[hh].append(("full", ew))
                    # diagonal tiles D1, D2
                    for d in range(2):
                        qw = QKP.tile([128, 2 * CHUNK], fp32, tag="qk",
                                      name="qwd")
                        for k in range(2):
                            i = 2 * d + k
                            off, w = D_OFF[i]
                            jb = 4 * c + i
                            js = slice(128 * jb, 128 * jb + 128)
                            sis = slice(CHUNK * c + 128 * i,
                                        CHUNK * (c + 1))
                            nc.tensor.matmul(
                                qw[:, off:off + w],
                                kt2[qsl, js], qt[mt][qsl, sis],
                                start=True, stop=True,
                                tile_position=(pbase, 0))
                        ew = EP.tile([128, 2 * CHUNK], bfl, tag="e", name="ewd")
                        wtot = D_OFF[2 * d][1] + D_OFF[2 * d + 1][1]
                        nc.scalar.activation(ew[:, 0:wtot], qw[:, 0:wtot],
                                             Exp, scale=0.125)
                        # triangle mask: one op covers the leading 128 cols
                        # of both jbs via a [128, 2, 128] strided view
                        half = D_OFF[2 * d + 1][0]  # 512 for D1, 256 for D2
                        mv = ew[:, 0:2 * half].rearrange(
                            "p (a c) -> p a c", a=2)[:, :, 0:128]
                        nc.vector.tensor_mul(mv, mv, tri_t[:])
                        if DBG and c == 0 and mt == 0 and hh == 0 and d == 0:
                            nc.sync.dma_start(dew.ap(), ew[:])
                        ews[hh].append(("diag", ew))
                return ews

            def emit_pv_div(c, mt, ews):
                cs = slice(CHUNK * c, CHUNK * (c + 1))
                njb = 4 * c + 4
                pv0 = PVP.tile([65, CHUNK], fp32, tag="pv", name="pv0")
                pv1 = PVP.tile([65, CHUNK], fp32, tag="pv", name="pv1")
                for hh, pvt in ((0, pv0), (1, pv1)):
                    for p, (kind, ew) in enumerate(ews[hh]):
                        if kind == "full":
                            for i, jb in enumerate((2 * p, 2 * p + 1)):
                                nc.tensor.matmul(
                                    pvt[:, :], vones[jb][:, 0:65],
                                    ew[:, CHUNK * i:CHUNK * (i + 1)],
                                    start=(jb == 0), stop=False)
                        else:
                            d = p - 2 * c
                            for k in range(2):
                                i = 2 * d + k
                                off, w = D_OFF[i]
                                jb = 4 * c + i
                                nc.tensor.matmul(
                                    pvt[:, 128 * i:CHUNK],
                                    vones[jb][:, 0:65],
                                    ew[:, off:off + w],
                                    start=(jb == 0), stop=(jb == njb - 1))
                for hh, pv in ((0, pv0), (1, pv1)):
                    rf = SM.tile([1, CHUNK], fp32, tag="rf", name="rf")
                    nc.vector.reciprocal(rf[:], pv[64:65, :])
                    bcs = SM.tile([64, CHUNK], fp32, tag="bcs", name="bcs")
                    nc.gpsimd.partition_broadcast(bcs[:], rf[:])
                    if DBG and c == 0 and mt == 0 and hh == 0:
                        pvs = SM.tile([65, CHUNK], fp32, tag="pvs", name="pvs")
                        nc.vector.tensor_copy(pvs[:], pv[:])
                        nc.sync.dma_start(dpv.ap(), pvs[:])
                        nc.sync.dma_start(drf.ap(), rf[:])
                        nc.sync.dma_start(dbcs.ap(), bcs[:])
                    nc.vector.tensor_mul(at[mt][64 * hh:64 * hh + 64, cs],
                                         pv[0:64, :], bcs[:])

            def emit_oproj(c):
                for st in range(4):
                    sit = 4 * c + st
                    ss = slice(128 * sit, 128 * sit + 128)
                    ob = OB.tile([128, H], bfl, tag="ob", name="ob")
                    for oc in range(4):
                        po = PS5.tile([128, CHUNK], fp32, tag="p5", name="po")
                        for mt in range(nMT):
                            nc.tensor.matmul(
                                po[:], at[mt][:, ss],
                                wo_t[mt][:, CHUNK * oc:CHUNK * (oc + 1)],
                                start=(mt == 0), stop=(mt == nMT - 1))
                        nc.vector.tensor_copy(
                            ob[:, CHUNK * oc:CHUNK * (oc + 1)], po[:])
                    nc.sync.dma_start(out.ap()[ss, :], ob[:])

            # ---------------- pipeline ----------------------------------
            # Projections of chunk c+1 are emitted interleaved with the
            # attention groups of chunk c; QK/exp of group i+1 is emitted
            # before PV/divide of group i so ACT stays fed while PE drains.
            emit_kv(0)
            if DBG:
                nc.sync.dma_start(dkt2a.ap(), kt2[:])
                nc.sync.dma_start(dvoa.ap(), vones[0][:])
            for mt in range(nMT):
                emit_q(0, mt)
            groups = [(c, mt) for c in range(nC) for mt in range(nMT)]
            pend = emit_qk(*groups[0])
            for i, (c, mt) in enumerate(groups):
                if mt == 0 and c + 1 < nC:
                    emit_kv(c + 1)
                if mt == 2 and c + 2 < nC:
                    emit_x(c + 2)
                nxt = emit_qk(*groups[i + 1]) if i + 1 < len(groups) else None
                if c + 1 < nC:
                    emit_q(c + 1, mt)
                emit_pv_div(c, mt, pend)
                if mt == nMT - 1:
                    emit_oproj(c)
                pend = nxt
            if DBG:
                nc.sync.dma_start(dkt2.ap(), kt2[:])
                nc.sync.dma_start(dqt0.ap(), qt[0][:])
                nc.sync.dma_start(dvt.ap(), vt_sb[:])
                nc.sync.dma_start(dvo0.ap(), vones[0][:])
                nc.sync.dma_start(dat0.ap(), at[0][:])
    return nc


# ---------------------------------------------------------------------------
# host side
# ---------------------------------------------------------------------------

def _rope_tables(seq):
    """Tables in the PERM64 row layout: row 2i holds dim i, row 2i+1 holds
    dim i+32 (pairs adjacent).  cos'[r] = cos[i]; sin'[2i] = +sin[i],
    sin'[2i+1] = -sin[i]: after the adjacent-row swap of rs = t0*sin',
    out[2i] = q[i]cos[i] - q[i+32]sin[i], out[2i+1] = q[i+32]cos[i] +
    q[i]sin[i], matching rotate_half RoPE."""
    inv_freq = 1.0 / (ROPE_THETA ** (np.arange(0, HEAD_DIM, 2, dtype=np.float32)
                                     / HEAD_DIM))
    t = np.arange(seq, dtype=np.float32)
    freqs = np.outer(t, inv_freq)                       # [S, 32] (dim i)
    cosv = np.cos(freqs).astype(np.float32)             # [S, 32]
    sinv = np.sin(freqs).astype(np.float32)
    cosP = np.empty((HEAD_DIM, seq), np.float32)
    sinP = np.empty((HEAD_DIM, seq), np.float32)
    for i in range(32):
        cosP[2 * i] = cosP[2 * i + 1] = cosv[:, i]
        sinP[2 * i] = sinv[:, i]
        sinP[2 * i + 1] = -sinv[:, i]
    cos2 = np.tile(cosP, (2, 1)).copy()                 # [128, S]
    sinS = np.tile(sinP, (2, 1)).copy()
    return cos2, sinS


def _perm_rope_rows(w, nheads):
    """Permute rope'd projection rows (q/k) into the PERM64 layout:
    out row 64h + r = in row 64h + PERM64[r]."""
    w = w.reshape(nheads, HEAD_DIM, *w.shape[1:])
    return np.ascontiguousarray(w[:, PERM64].reshape(nheads * HEAD_DIM,
                                                     *w.shape[2:]))


def host_inputs(x, Wq, bq, Wk, bk, Wv, bv, Wo, seq=S):
    cos2, sinS = _rope_tables(seq)
    cos2 = cos2.astype(bf16)
    sinS = sinS.astype(bf16)
    r = np.arange(128)
    # tri[sj_row, si_col] keeps si >= sj within the diagonal 128-block,
    # duplicated for the [128, 2, 128] strided mask views
    tri1 = (r[None, :128] >= r[:, None]).astype(np.float32).astype(bf16)
    tri = np.ascontiguousarray(np.stack([tri1, tri1], axis=1))
    id64 = np.eye(64, dtype=np.float32)
    xTb = [np.ascontiguousarray(x[b, :seq, :].T).astype(bf16) for b in range(B)]
    in_maps = []
    for d in range(N_CORES):
        b, g = d // 4, d % 4
        wq_p = _perm_rope_rows(Wq[ML * g:ML * (g + 1), :], 8)
        bq_p = _perm_rope_rows(bq[ML * g:ML * (g + 1)], 8)
        wk_p = _perm_rope_rows(Wk[64 * g:64 * (g + 1), :], 1)
        bk_p = _perm_rope_rows(bk[64 * g:64 * (g + 1)], 1)
        wq_s = np.ascontiguousarray(wq_p.T).astype(bf16)
        wk_s = np.ascontiguousarray(wk_p.T).astype(bf16)
        wv_s = np.ascontiguousarray(Wv[64 * g:64 * (g + 1), :].T).astype(bf16)
        wkv_s = np.concatenate([wk_s, wv_s], axis=1)
        wo_s = np.ascontiguousarray(Wo[:, ML * g:ML * (g + 1)].T).astype(bf16)
        bq_s = np.ascontiguousarray(
            bq_p.reshape(4, 128).T).astype(np.float32)
        bkv_s = np.concatenate([bk_p,
                                bv[64 * g:64 * (g + 1)]]).reshape(128, 1)
        in_maps.append({
            "xT": xTb[b], "wq": wq_s, "wkv": wkv_s, "wo": wo_s,
            "bq": bq_s, "bkv": np.ascontiguousarray(bkv_s, dtype=np.float32),
            "cos2": cos2[:, :seq], "sinS": sinS[:, :seq], "tri": tri,
            "id64": id64,
        })
    return in_maps


_NC = None


def _get_nc():
    global _NC
    if _NC is None:
        import concourse.tile as tile_mod
        from concourse import bacc, mybir
        nc = bacc.Bacc("TRN2", target_bir_lowering=False, debug=False,
                       num_devices=N_CORES)
        build_graph(nc, tile_mod, mybir)
        nc.compile()
        _NC = nc
    return _NC


def kernel(**inputs):
    from concourse import bass_utils
    nc = _get_nc()
    x = np.asarray(inputs["x"], dtype=np.float32)
    in_maps = host_inputs(
        x, np.asarray(inputs["Wq"], np.float32), np.asarray(inputs["bq"], np.float32),
        np.asarray(inputs["Wk"], np.float32), np.asarray(inputs["bk"], np.float32),
        np.asarray(inputs["Wv"], np.float32), np.asarray(inputs["bv"], np.float32),
        np.asarray(inputs["Wo"], np.float32))
    res = bass_utils.run_bass_kernel_spmd(nc, in_maps, core_ids=list(range(N_CORES)))
    bo = np.asarray(inputs["bo"], np.float32)
    out = np.empty((B, S, H), dtype=np.float32)
    for b in range(B):
        acc = res.results[4 * b]["out"].astype(np.float32).copy()
        for g in range(1, 4):
            acc += res.results[4 * b + g]["out"]
        out[b] = acc + bo[None, :]
    return out


# revision 43
# speedup vs baseline: 2.1371x; 2.1371x over previous
"""GQA attention (RoPE, causal) for one TRN2 chip (8 NeuronCores).

Sharding: core d handles batch b = d//4 and kv-group g = d%4
(8 q heads + 1 kv head per core).  Each core computes its partial
output-projection contribution out_partial[b] (shape [S, H]); the host
sums the 4 partials per batch and adds bo.  No collectives.

Per-core layout (all matmul contractions on partitions, all bf16):
  xT   [H, S]       : x[b]^T, loaded chunk-by-chunk (si) double-buffered
  QT   [512, S]     : q^T head-major (rope'd), PERM64 row layout
  KT2  [128, S]     : k^T rope'd (PERM64 rows), duplicated in both halves
  V    [sj,64]+1s   : v transposed with an appended ones column
  scores^T [sj,si]  : lhsT=KT2 block, rhs=QT chunk; diagonal blocks get
                      trimmed si widths packed into two psum tiles
  exp (ACT, scale=1/8; no max-subtraction: |s/8| <~ 6 so exp is safe)
  causal: triangle mask multiply on diag blocks only (DVE, strided view)
  PV: lhsT=[V|1] [sj,65], rhs=expS^T -> psum [65, si] = [attn^T; denom]
  divide: reciprocal_approx (DVE) -> partition_broadcast (Pool) -> mul
  out[s,o]: lhsT=attnT tiles, rhs=woT tiles, psum->sbuf copy, DMA out.

RoPE trick: head-dim rows are stored in PERM64 order (pair (d, d+32) in
adjacent rows), so rotate-half is an adjacent-row swap that a single
DVE stream_shuffle can do (shuffles are quadrant-local).  Wq/Wk rows,
bq/bk, and the cos/sin tables are permuted on the host; QK dot products
are invariant since q and k share the permutation.
"""

import sys

if "/opt/trn_rl_repo" not in sys.path:
    sys.path.insert(0, "/opt/trn_rl_repo")

import numpy as np
import ml_dtypes

bf16 = ml_dtypes.bfloat16

B = 2
S = 2048
H = 2048
N_HEADS = 32
KV_HEADS = 4
HEAD_DIM = 64
ROPE_THETA = 10000.0
N_CORES = 8
ML = 512          # q-head features per core (8 heads * 64)
CHUNK = 512       # si chunk width
HB = 128          # contraction tile

SWAPADJ = [g ^ 1 for g in range(32)]
PERM64 = [(r >> 1) + 32 * (r & 1) for r in range(64)]   # row r <- dim d(r)


def build_graph(nc, tile_mod, mybir, seq=S):
    fp32 = mybir.dt.float32
    bfl = mybir.dt.bfloat16

    nC = seq // CHUNK       # si chunks
    nJ = seq // 128         # sj blocks
    nHB = H // HB           # contraction tiles
    nMT = ML // 128         # q-feature partition tiles (head pairs)

    xT = nc.dram_tensor("xT", [H, seq], bfl, kind="ExternalInput")
    wq = nc.dram_tensor("wq", [H, ML], bfl, kind="ExternalInput")
    wkv = nc.dram_tensor("wkv", [H, 128], bfl, kind="ExternalInput")
    wo = nc.dram_tensor("wo", [ML, H], bfl, kind="ExternalInput")
    bq = nc.dram_tensor("bq", [128, nMT], fp32, kind="ExternalInput")
    bkv = nc.dram_tensor("bkv", [128, 1], fp32, kind="ExternalInput")
    cos2 = nc.dram_tensor("cos2", [128, seq], bfl, kind="ExternalInput")
    sinS = nc.dram_tensor("sinS", [128, seq], bfl, kind="ExternalInput")
    tri = nc.dram_tensor("tri", [128, 2, 128], bfl, kind="ExternalInput")
    id64 = nc.dram_tensor("id64", [64, 64], fp32, kind="ExternalInput")
    out = nc.dram_tensor("out", [seq, H], bfl, kind="ExternalOutput")
    DBG = getattr(build_graph, "_debug", False)
    if DBG:
        dkt2 = nc.dram_tensor("dkt2", [128, seq], bfl, kind="ExternalOutput")
        dqt0 = nc.dram_tensor("dqt0", [128, seq], bfl, kind="ExternalOutput")
        dvt = nc.dram_tensor("dvt", [64, seq], fp32, kind="ExternalOutput")
        dvo0 = nc.dram_tensor("dvo0", [128, 65], bfl, kind="ExternalOutput")
        dat0 = nc.dram_tensor("dat0", [128, seq], bfl, kind="ExternalOutput")
        dew = nc.dram_tensor("dew", [128, 1024], bfl, kind="ExternalOutput")
        dkt2a = nc.dram_tensor("dkt2a", [128, seq], bfl, kind="ExternalOutput")
        dvoa = nc.dram_tensor("dvoa", [128, 65], bfl, kind="ExternalOutput")
        dpv = nc.dram_tensor("dpv", [65, 512], fp32, kind="ExternalOutput")
        drf = nc.dram_tensor("drf", [1, 512], fp32, kind="ExternalOutput")
        dbcs = nc.dram_tensor("dbcs", [64, 512], fp32, kind="ExternalOutput")

    Exp = mybir.ActivationFunctionType.Exp
    tc = tile_mod.TileContext(nc)
    with tc:
        with tc.tile_pool(name="persist", bufs=1) as P, \
             tc.tile_pool(name="xpool", bufs=2) as XP, \
             tc.tile_pool(name="qkps", bufs=2, space="PSUM") as QKP, \
             tc.tile_pool(name="pvps", bufs=2, space="PSUM") as PVP, \
             tc.tile_pool(name="ps512", bufs=2, space="PSUM") as PS5, \
             tc.tile_pool(name="expp", bufs=16) as EP, \
             tc.tile_pool(name="small", bufs=4) as SM, \
             tc.tile_pool(name="rope", bufs=3) as RP, \
             tc.tile_pool(name="outb", bufs=3) as OB:
            wq_t = [P.tile([128, ML], bfl, tag=f"wq{h}", name=f"wq{h}")
                    for h in range(nHB)]
            wkv_t = [P.tile([128, 128], bfl, tag=f"wkv{h}", name=f"wkv{h}")
                     for h in range(nHB)]
            wo_t = [P.tile([128, H], bfl, tag=f"wo{mt}", name=f"wo{mt}")
                    for mt in range(nMT)]
            qt = [P.tile([128, seq], bfl, tag=f"qt{mt}", name=f"qt{mt}")
                  for mt in range(nMT)]
            at = [P.tile([128, seq], bfl, tag=f"at{mt}", name=f"at{mt}")
                  for mt in range(nMT)]
            kt2 = P.tile([128, seq], bfl, tag="kt2", name="kt2")
            vt_sb = P.tile([64, seq], fp32, tag="vt", name="vt_sb")
            vones = [P.tile([128, 65], bfl, tag=f"vo{j}", name=f"vo{j}")
                     for j in range(nJ)]
            cos_t = P.tile([128, seq], bfl, tag="cos", name="cos_t")
            sin_t = P.tile([128, seq], bfl, tag="sin", name="sin_t")
            tri_t = P.tile([128, 2, 128], bfl, tag="tri", name="tri_t")
            bq_t = P.tile([128, nMT], fp32, tag="bq", name="bq_t")
            bkv_t = P.tile([128, 1], fp32, tag="bkv", name="bkv_t")
            id64_t = P.tile([64, 64], fp32, tag="id64", name="id64_t")

            x_tiles = {}

            def emit_x(c):
                cs = slice(CHUNK * c, CHUNK * (c + 1))
                tiles = []
                for h in range(nHB):
                    t = XP.tile([128, CHUNK], bfl, tag=f"x{h}", name=f"x{h}")
                    nc.sync.dma_start(t[:], xT.ap()[128 * h:128 * h + 128, cs])
                    tiles.append(t)
                x_tiles[c] = tiles

            # ---- input DMAs (front-loaded) ------------------------------
            for h in range(nHB):
                nc.sync.dma_start(wkv_t[h][:], wkv.ap()[128 * h:128 * h + 128, :])
            emit_x(0)
            nc.sync.dma_start(cos_t[:], cos2.ap())
            nc.sync.dma_start(sin_t[:], sinS.ap())
            nc.sync.dma_start(bkv_t[:], bkv.ap())
            nc.sync.dma_start(bq_t[:], bq.ap())
            nc.sync.dma_start(tri_t[:], tri.ap())
            nc.sync.dma_start(id64_t[:], id64.ap())
            for h in range(nHB):
                nc.sync.dma_start(wq_t[h][:], wq.ap()[128 * h:128 * h + 128, :])
            if nC > 1:
                emit_x(1)
            for mt in range(nMT):
                nc.sync.dma_start(wo_t[mt][:], wo.ap()[128 * mt:128 * mt + 128, :])

            def rope(ps, bias_col, cs, dst, nrows):
                """RoPE a [128,CHUNK] psum tile into dst (bf16).
                nrows=128 ropes both 64-blocks (Q); nrows=64 ropes rows
                0:64 only (K; rows 64:128 pass through as V via t0)."""
                t0 = RP.tile([128, CHUNK], bfl, tag="t0", name="t0")
                nc.vector.tensor_scalar_add(t0[:], ps[:], bias_col)
                rs = RP.tile([128, CHUNK], bfl, tag="rs", name="rs")
                nc.vector.tensor_mul(rs[0:nrows, :], t0[0:nrows, :],
                                     sin_t[0:nrows, cs])
                sh = RP.tile([128, CHUNK], bfl, tag="sh", name="sh")
                nc.vector.stream_shuffle(sh[0:nrows, :], rs[0:nrows, :],
                                         SWAPADJ)
                m1 = RP.tile([128, CHUNK], bfl, tag="m1", name="m1")
                nc.vector.tensor_mul(m1[0:nrows, :], t0[0:nrows, :],
                                     cos_t[0:nrows, cs])
                nc.vector.tensor_add(dst, m1[0:nrows, :], sh[0:nrows, :])
                return t0

            def emit_kv(c):
                cs = slice(CHUNK * c, CHUNK * (c + 1))
                xc = x_tiles[c]
                ps = PS5.tile([128, CHUNK], fp32, tag="p5", name="pskv")
                for h in range(nHB):
                    nc.tensor.matmul(ps[:], wkv_t[h][:], xc[h][:],
                                     start=(h == 0), stop=(h == nHB - 1))
                t0 = rope(ps, bkv_t[:, 0:1], cs, kt2[0:64, cs], 64)
                nc.gpsimd.tensor_copy(kt2[64:128, cs], kt2[0:64, cs])
                nc.gpsimd.tensor_copy(vt_sb[:, cs], t0[64:128, :])
                # V transposes for this chunk's sj blocks (fp32 so the
                # transpose can share the fp32 psum pool)
                for j in range(4 * c, 4 * c + 4):
                    pv = PS5.tile([128, CHUNK], fp32, tag="p5", name="psv")
                    nc.tensor.transpose(pv[:, 0:64],
                                        vt_sb[:, 128 * j:128 * j + 128],
                                        id64_t[:])
                    nc.gpsimd.memset(vones[j][:, 64:65], 1.0)
                    nc.scalar.copy(vones[j][:, 0:64], pv[:, 0:64])

            def emit_q(c, mt):
                cs = slice(CHUNK * c, CHUNK * (c + 1))
                ms = slice(128 * mt, 128 * mt + 128)
                xc = x_tiles[c]
                ps = PS5.tile([128, CHUNK], fp32, tag="p5", name="psq")
                for h in range(nHB):
                    nc.tensor.matmul(ps[:], wq_t[h][:, ms], xc[h][:],
                                     start=(h == 0), stop=(h == nHB - 1))
                rope(ps, bq_t[:, mt:mt + 1], cs, qt[mt][:, cs], 128)

            # diag pack: D1 holds jb=4c+0 (w=512 @0) and 4c+1 (w=384 @512);
            # D2 holds jb=4c+2 (w=256 @0) and 4c+3 (w=128 @256).
            D_OFF = [(0, 512), (512, 384), (0, 256), (256, 128)]

            def emit_qk(c, mt):
                """QK matmuls + exp (+ diag masks) for both heads of mt.
                Returns per-hh list of (kind, ew) tiles."""
                cs = slice(CHUNK * c, CHUNK * (c + 1))
                ews = [[], []]
                for hh, pbase in ((0, 0), (1, 64)):
                    qsl = slice(pbase, pbase + 64)
                    for p in range(2 * c):   # off-diagonal pair tiles
                        qw = QKP.tile([128, 2 * CHUNK], fp32, tag="qk",
                                      name="qw")
                        for i, jb in enumerate((2 * p, 2 * p + 1)):
                            js = slice(128 * jb, 128 * jb + 128)
                            nc.tensor.matmul(
                                qw[:, CHUNK * i:CHUNK * (i + 1)],
                                kt2[qsl, js], qt[mt][qsl, cs],
                                start=True, stop=True,
                                tile_position=(pbase, 0))
                        ew = EP.tile([128, 2 * CHUNK], bfl, tag="e", name="ew")
                        nc.scalar.activation(ew[:], qw[:], Exp, scale=0.125)
                        ews[hh].append(("full", ew))
                    # diagonal tiles D1, D2
                    for d in range(2):
                        qw = QKP.tile([128, 2 * CHUNK], fp32, tag="qk",
                                      name="qwd")
                        for k in range(2):
                            i = 2 * d + k
                            off, w = D_OFF[i]
                            jb = 4 * c + i
                            js = slice(128 * jb, 128 * jb + 128)
                            sis = slice(CHUNK * c + 128 * i,
                                        CHUNK * (c + 1))
                            nc.tensor.matmul(
                                qw[:, off:off + w],
                                kt2[qsl, js], qt[mt][qsl, sis],
                                start=True, stop=True,
                                tile_position=(pbase, 0))
                        ew = EP.tile([128, 2 * CHUNK], bfl, tag="e", name="ewd")
                        wtot = D_OFF[2 * d][1] + D_OFF[2 * d + 1][1]
                        nc.scalar.activation(ew[:, 0:wtot], qw[:, 0:wtot],
                                             Exp, scale=0.125)
                        # triangle mask: one op covers the leading 128 cols
                        # of both jbs via a [128, 2, 128] strided view
                        half = D_OFF[2 * d + 1][0]  # 512 for D1, 256 for D2
                        mv = ew[:, 0:2 * half].rearrange(
                            "p (a c) -> p a c", a=2)[:, :, 0:128]
                        nc.vector.tensor_mul(mv, mv, tri_t[:])
                        if DBG and c == 0 and mt == 0 and hh == 0 and d == 0:
                            nc.sync.dma_start(dew.ap(), ew[:])
                        ews[hh].append(("diag", ew))
                return ews

            def emit_pv_div(c, mt, ews):
                cs = slice(CHUNK * c, CHUNK * (c + 1))
                njb = 4 * c + 4
                pv0 = PVP.tile([65, CHUNK], fp32, tag="pv", name="pv0")
                pv1 = PVP.tile([65, CHUNK], fp32, tag="pv", name="pv1")
                for hh, pvt in ((0, pv0), (1, pv1)):
                    for p, (kind, ew) in enumerate(ews[hh]):
                        if kind == "full":
                            for i, jb in enumerate((2 * p, 2 * p + 1)):
                                nc.tensor.matmul(
                                    pvt[:, :], vones[jb][:, 0:65],
                                    ew[:, CHUNK * i:CHUNK * (i + 1)],
                                    start=(jb == 0), stop=False)
                        else:
                            d = p - 2 * c
                            for k in range(2):
                                i = 2 * d + k
                                off, w = D_OFF[i]
                                jb = 4 * c + i
                                nc.tensor.matmul(
                                    pvt[:, 128 * i:CHUNK],
                                    vones[jb][:, 0:65],
                                    ew[:, off:off + w],
                                    start=(jb == 0), stop=(jb == njb - 1))
                for hh, pv in ((0, pv0), (1, pv1)):
                    rf = SM.tile([1, CHUNK], fp32, tag="rf", name="rf")
                    nc.vector.reciprocal(rf[:], pv[64:65, :])
                    bcs = SM.tile([64, CHUNK], fp32, tag="bcs", name="bcs")
                    nc.gpsimd.partition_broadcast(bcs[:], rf[:])
                    if DBG and c == 0 and mt == 0 and hh == 0:
                        pvs = SM.tile([65, CHUNK], fp32, tag="pvs", name="pvs")
                        nc.vector.tensor_copy(pvs[:], pv[:])
                        nc.sync.dma_start(dpv.ap(), pvs[:])
                        nc.sync.dma_start(drf.ap(), rf[:])
                        nc.sync.dma_start(dbcs.ap(), bcs[:])
                    nc.vector.tensor_mul(at[mt][64 * hh:64 * hh + 64, cs],
                                         pv[0:64, :], bcs[:])

            def emit_oproj(c):
                for st in range(4):
                    sit = 4 * c + st
                    ss = slice(128 * sit, 128 * sit + 128)
                    ob = OB.tile([128, H], bfl, tag="ob", name="ob")
                    for oc in range(4):
                        po = PS5.tile([128, CHUNK], fp32, tag="p5", name="po")
                        for mt in range(nMT):
                            nc.tensor.matmul(
                                po[:], at[mt][:, ss],
                                wo_t[mt][:, CHUNK * oc:CHUNK * (oc + 1)],
                                start=(mt == 0), stop=(mt == nMT - 1))
                        nc.vector.tensor_copy(
                            ob[:, CHUNK * oc:CHUNK * (oc + 1)], po[:])
                    nc.sync.dma_start(out.ap()[ss, :], ob[:])

            # ---------------- pipeline ----------------------------------
            # Projections of chunk c+1 are emitted interleaved with the
            # attention groups of chunk c; QK/exp of group i+1 is emitted
            # before PV/divide of group i so ACT stays fed while PE drains.
            emit_kv(0)
            if DBG:
                nc.sync.dma_start(dkt2a.ap(), kt2[:])
                nc.sync.dma_start(dvoa.ap(), vones[0][:])
            for mt in range(nMT):
                emit_q(0, mt)
            groups = [(c, mt) for c in range(nC) for mt in range(nMT)]
            pend = emit_qk(*groups[0])
            for i, (c, mt) in enumerate(groups):
                if mt == 0 and c + 1 < nC:
                    emit_kv(c + 1)
                if mt == 2 and c + 2 < nC:
                    emit_x(c + 2)
                nxt = emit_qk(*groups[i + 1]) if i + 1 < len(groups) else None
                if c + 1 < nC:
                    emit_q(c + 1, mt)
                emit_pv_div(c, mt, pend)
                if mt == nMT - 1:
                    emit_oproj(c)
                pend = nxt
            if DBG:
                nc.sync.dma_start(dkt2.ap(), kt2[:])
                nc.sync.dma_start(dqt0.ap(), qt[0][:])
                nc.sync.dma_start(dvt.ap(), vt_sb[:])
                nc.sync.dma_start(dvo0.ap(), vones[0][:])
                nc.sync.dma_start(dat0.ap(), at[0][:])
    return nc


# ---------------------------------------------------------------------------
# host side
# ---------------------------------------------------------------------------

def _rope_tables(seq):
    """Tables in the PERM64 row layout: row 2i holds dim i, row 2i+1 holds
    dim i+32 (pairs adjacent).  cos'[r] = cos[i]; sin'[2i] = +sin[i],
    sin'[2i+1] = -sin[i]: after the adjacent-row swap of rs = t0*sin',
    out[2i] = q[i]cos[i] - q[i+32]sin[i], out[2i+1] = q[i+32]cos[i] +
    q[i]sin[i], matching rotate_half RoPE."""
    inv_freq = 1.0 / (ROPE_THETA ** (np.arange(0, HEAD_DIM, 2, dtype=np.float32)
                                     / HEAD_DIM))
    t = np.arange(seq, dtype=np.float32)
    freqs = np.outer(t, inv_freq)                       # [S, 32] (dim i)
    cosv = np.cos(freqs).astype(np.float32)             # [S, 32]
    sinv = np.sin(freqs).astype(np.float32)
    cosP = np.empty((HEAD_DIM, seq), np.float32)
    sinP = np.empty((HEAD_DIM, seq), np.float32)
    for i in range(32):
        cosP[2 * i] = cosP[2 * i + 1] = cosv[:, i]
        sinP[2 * i] = sinv[:, i]
        sinP[2 * i + 1] = -sinv[:, i]
    cos2 = np.tile(cosP, (2, 1)).copy()                 # [128, S]
    sinS = np.tile(sinP, (2, 1)).copy()
    return cos2, sinS


def _perm_rope_rows(w, nheads):
    """Permute rope'd projection rows (q/k) into the PERM64 layout:
    out row 64h + r = in row 64h + PERM64[r]."""
    w = w.reshape(nheads, HEAD_DIM, *w.shape[1:])
    return np.ascontiguousarray(w[:, PERM64].reshape(nheads * HEAD_DIM,
                                                     *w.shape[2:]))


def host_inputs(x, Wq, bq, Wk, bk, Wv, bv, Wo, seq=S):
    cos2, sinS = _rope_tables(seq)
    cos2 = cos2.astype(bf16)
    sinS = sinS.astype(bf16)
    r = np.arange(128)
    # tri[sj_row, si_col] keeps si >= sj within the diagonal 128-block,
    # duplicated for the [128, 2, 128] strided mask views
    tri1 = (r[None, :128] >= r[:, None]).astype(np.float32).astype(bf16)
    tri = np.ascontiguousarray(np.stack([tri1, tri1], axis=1))
    id64 = np.eye(64, dtype=np.float32)
    xTb = [np.ascontiguousarray(x[b, :seq, :].T).astype(bf16) for b in range(B)]
    in_maps = []
    for d in range(N_CORES):
        b, g = d // 4, d % 4
        wq_p = _perm_rope_rows(Wq[ML * g:ML * (g + 1), :], 8)
        bq_p = _perm_rope_rows(bq[ML * g:ML * (g + 1)], 8)
        wk_p = _perm_rope_rows(Wk[64 * g:64 * (g + 1), :], 1)
        bk_p = _perm_rope_rows(bk[64 * g:64 * (g + 1)], 1)
        wq_s = np.ascontiguousarray(wq_p.T).astype(bf16)
        wk_s = np.ascontiguousarray(wk_p.T).astype(bf16)
        wv_s = np.ascontiguousarray(Wv[64 * g:64 * (g + 1), :].T).astype(bf16)
        wkv_s = np.concatenate([wk_s, wv_s], axis=1)
        wo_s = np.ascontiguousarray(Wo[:, ML * g:ML * (g + 1)].T).astype(bf16)
        bq_s = np.ascontiguousarray(
            bq_p.reshape(4, 128).T).astype(np.float32)
        bkv_s = np.concatenate([bk_p,
                                bv[64 * g:64 * (g + 1)]]).reshape(128, 1)
        in_maps.append({
            "xT": xTb[b], "wq": wq_s, "wkv": wkv_s, "wo": wo_s,
            "bq": bq_s, "bkv": np.ascontiguousarray(bkv_s, dtype=np.float32),
            "cos2": cos2[:, :seq], "sinS": sinS[:, :seq], "tri": tri,
            "id64": id64,
        })
    return in_maps


_NC = None


def _get_nc():
    global _NC
    if _NC is None:
        import concourse.tile as tile_mod
        from concourse import bacc, mybir
        nc = bacc.Bacc("TRN2", target_bir_lowering=False, debug=False,
                       num_devices=N_CORES)
        build_graph(nc, tile_mod, mybir)
        nc.compile()
        _NC = nc
    return _NC


def kernel(**inputs):
    from concourse import bass_utils
    nc = _get_nc()
    x = np.asarray(inputs["x"], dtype=np.float32)
    in_maps = host_inputs(
        x, np.asarray(inputs["Wq"], np.float32), np.asarray(inputs["bq"], np.float32),
        np.asarray(inputs["Wk"], np.float32), np.asarray(inputs["bk"], np.float32),
        np.asarray(inputs["Wv"], np.float32), np.asarray(inputs["bv"], np.float32),
        np.asarray(inputs["Wo"], np.float32))
    res = bass_utils.run_bass_kernel_spmd(nc, in_maps, core_ids=list(range(N_CORES)))
    bo = np.asarray(inputs["bo"], np.float32)
    out = np.empty((B, S, H), dtype=np.float32)
    for b in range(B):
        acc = res.results[4 * b]["out"].astype(np.float32).copy()
        for g in range(1, 4):
            acc += res.results[4 * b + g]["out"]
        out[b] = acc + bo[None, :]
    return out
